# revision 1
# baseline (speedup 1.0000x reference)
"""DeepSeekV2-style MLA decode attention (MQA, B=128 decode tokens) on 8 trn2 NeuronCores.

Strategy (all shapes hardcoded from the problem spec):
  - Heads tensor-parallel for the projections: core c owns 16 heads (Wq cols /
    Wo rows sharded); Wkv replicated (tiny).
  - Batch-parallel for attention: core c owns 16 of the 128 decode batches;
    k/v caches are sharded by batch, truncated to seq_len, and staged by the
    host (kT pre-transposed) so the device streams only valid positions.
  - Two on-device AllToAlls reshard q (head-shard -> batch-shard) and attn
    output (batch-shard -> head-shard).  Final Wo partials are summed on host.
  - New-token kv (h @ Wkv) is computed on-device; the new token occupies
    position 0 of each staged cache slot, so its patch location is static.
  - Everything enters the PE as bf16 (host-cast); accumulation is f32.

The device program is specialized on seq_lens (static loop bounds); kernel()
re-builds/compiles whenever the derived schedule changes and memoizes it.
"""

import os
import numpy as np
import ml_dtypes

import concourse.bass as bass
import concourse.mybir as mybir
import concourse.tile as tile
from concourse import bacc
from concourse.bass_utils import run_bass_kernel_spmd

BF16 = ml_dtypes.bfloat16
P = 128
B, MAX_S, HID = 128, 4096, 5120
H, D, D_ROPE, D_V = 128, 128, 64, 128
NC, HPC, BPC = 8, 16, 16           # cores, heads/core, slots(batches)/core
QCOLS = HPC * D + 256              # 2304: per-core q cols + kv cols
SCALE = float(D) ** -0.5
PROJ_CHUNKS = [(0, 512), (512, 512), (1024, 512), (1536, 512), (2048, 256)]
WO_PASSES = [(0, 1024), (1024, 1024), (2048, 1024), (3072, 1024), (4096, 1024)]

dtb = mybir.dt.bfloat16
dtf = mybir.dt.float32

_PROGRAM_CACHE: dict = {}
LAST_RESULTS = None


def _install_ntff_hook():
    """The agent image's ``antenv`` lacks ``axon_hooks``; recreate it so
    run_bass_kernel_spmd(trace=True) can capture NTFF profiles via the
    libaxon ctypes path. Best-effort — failure just means no trace."""
    import sys
    import types
    try:
        import antenv.axon_hooks  # noqa: F401
        return
    except ImportError:
        pass
    try:
        from trn_agent_boot.trn_boot import _ntff_profile_via_ctypes
        hook = _ntff_profile_via_ctypes("/opt/axon/libaxon_pjrt.so")
        mod = types.ModuleType("antenv.axon_hooks")
        mod._hook = hook
        mod.set_axon_ntff_profile_hook = lambda h: setattr(mod, "_hook", h)
        mod.get_axon_ntff_profile_hook = lambda: mod._hook
        sys.modules["antenv.axon_hooks"] = mod
        import antenv
        antenv.axon_hooks = mod
    except Exception:
        pass


def _build_program(T, toff, NTT):
    """Build + compile the per-core bass program. T/toff: per-slot tile counts
    and tile offsets (static schedule, same on every core). NTT = sum(T)."""
    nc = bacc.Bacc("TRN2", target_bir_lowering=False, debug=False, num_devices=NC)

    ht_d = nc.dram_tensor("ht", [P, HID], dtb, kind="ExternalInput")
    wq_d = nc.dram_tensor("wq", [HID, QCOLS], dtb, kind="ExternalInput")
    kt_d = nc.dram_tensor("kt", [P, NTT * P], dtb, kind="ExternalInput")
    vv_d = nc.dram_tensor("vv", [P, NTT * P], dtb, kind="ExternalInput")
    bias_d = nc.dram_tensor("bias", [P, NTT], dtf, kind="ExternalInput")
    wo_d = nc.dram_tensor("wo", [HPC * D_V, HID], dtb, kind="ExternalInput")
    idn_d = nc.dram_tensor("idn", [P, P], dtb, kind="ExternalInput")
    one_d = nc.dram_tensor("one", [P, 1], dtb, kind="ExternalInput")
    out_d = nc.dram_tensor("outp", [P, HID], dtf, kind="ExternalOutput")
    debug = os.environ.get("BASS_KERNEL_DEBUG", "0") == "1"
    if debug:
        dbg_qin = nc.dram_tensor("dbg_qin", [P, QCOLS], dtb, kind="ExternalOutput")
        dbg_qout = nc.dram_tensor("dbg_qout", [P, QCOLS], dtb, kind="ExternalOutput")
        dbg_at = nc.dram_tensor("dbg_at", [P, BPC * D_V], dtb, kind="ExternalOutput")
        dbg_ao = nc.dram_tensor("dbg_ao", [P, BPC * D_V], dtb, kind="ExternalOutput")
        dbg_sum = nc.dram_tensor("dbg_sum", [P, BPC], dtf, kind="ExternalOutput")
        dbg_et = nc.dram_tensor("dbg_et", [P, NTT * P], dtb, kind="ExternalOutput")
        dbg_qjt = nc.dram_tensor("dbg_qjt", [P, BPC * P], dtb, kind="ExternalOutput")
        dbg_kt = nc.dram_tensor("dbg_kt", [P, BPC * P], dtb, kind="ExternalOutput")

    rg = [list(range(NC))]
    Exp = mybir.ActivationFunctionType.Exp
    Copy = mybir.ActivationFunctionType.Copy

    with tile.TileContext(nc) as tc:
        with (
            tc.tile_pool(name="cpool", bufs=1) as cpool,
            tc.tile_pool(name="wpool", bufs=3) as wpool,
            tc.tile_pool(name="kvpool", bufs=2) as kvpool,
            tc.tile_pool(name="spool", bufs=3) as spool,
            tc.tile_pool(name="epool", bufs=6) as epool,
            tc.tile_pool(name="lhpool", bufs=HPC) as lhpool,
            tc.tile_pool(name="opool", bufs=2) as opool,
            tc.tile_pool(name="dram", bufs=1, space="DRAM") as dram,
        ):
            # ---- constants / global loads ----
            idn = cpool.tile([P, P], dtb)
            nc.sync.dma_start(idn[:], idn_d.ap())
            ones = cpool.tile([P, 1], dtb)
            nc.sync.dma_start(ones[:], one_d.ap())
            hts = cpool.tile([P, HID], dtb)
            nc.sync.dma_start(hts[:], ht_d.ap())
            bias_sb = cpool.tile([P, NTT], dtf)
            nc.sync.dma_start(bias_sb[:], bias_d.ap())

            qcc_in = dram.tile([P, QCOLS], dtb)
            qcc_out = dram.tile([P, QCOLS], dtb)
            acc_in = dram.tile([P, BPC * D_V], dtb)
            acc_out = dram.tile([P, BPC * D_V], dtb)

            # ---- phase A: q/kv projection (this core's 16 heads, all 128 batches) ----
            with tc.tile_pool(name="pjp", bufs=1, space="PSUM") as pjp:
                q_ps = pjp.tile([P, QCOLS], dtf)
                for k in range(HID // P):
                    wt = wpool.tile([P, QCOLS], dtb, tag="wt")
                    nc.sync.dma_start(wt[:], wq_d.ap()[k * P:(k + 1) * P, :])
                    for (c0, cw) in PROJ_CHUNKS:
                        nc.tensor.matmul(
                            q_ps[:, c0:c0 + cw],
                            lhsT=hts[:, k * P:(k + 1) * P],
                            rhs=wt[:, c0:c0 + cw],
                            start=(k == 0),
                            stop=(k == HID // P - 1),
                        )
                qc_sb = cpool.tile([P, QCOLS], dtb)
                nc.vector.tensor_copy(qc_sb[:], q_ps[:])

            nc.sync.dma_start(qcc_in[:], qc_sb[:])
            nc.gpsimd.collective_compute(
                "AllToAll", mybir.AluOpType.bypass, replica_groups=rg,
                ins=[qcc_in.opt()], outs=[qcc_out.opt()],
            )
            if debug:
                nc.sync.dma_start(dbg_qin.ap(), qc_sb[:])
                nc.sync.dma_start(dbg_qout.ap(), qcc_out[:])

            # ---- phase B: attention over this core's 16 batches ----
            attn_cc = cpool.tile([P, BPC * D_V], dtb)
            with (
                tc.tile_pool(name="tps", bufs=2, space="PSUM") as tps,
                tc.tile_pool(name="scps", bufs=2, space="PSUM") as scps,
                tc.tile_pool(name="aps", bufs=2, space="PSUM") as aps,
                tc.tile_pool(name="sps", bufs=2, space="PSUM") as sps,
            ):
                # new-token k/v from the AllToAll'd kv columns (identical from
                # every source core; read source block 0)
                knew = cpool.tile([BPC, P], dtb)
                nc.sync.dma_start(knew[:], qcc_out[0:BPC, HPC * D:HPC * D + P])
                knewT_ps = tps.tile([P, BPC], dtb, tag="tps")
                nc.tensor.transpose(knewT_ps[:], knew[:], idn[0:BPC, 0:BPC])
                knewT = cpool.tile([P, BPC], dtb)
                nc.vector.tensor_copy(knewT[:], knewT_ps[:])
                vnew = cpool.tile([1, BPC * P], dtb)
                nc.sync.dma_start(
                    vnew.rearrange("one (j d) -> one j d", d=P),
                    qcc_out.rearrange("(c j) f -> c j f", j=BPC)[0:1, :, HPC * D + P:QCOLS],
                )

                for j in range(BPC):
                    Tj = int(T[j])
                    # assemble q_j [128 h, 128 d] from gathered rows, then transpose
                    qj = spool.tile([P, P], dtb, tag="qj")
                    src = (
                        qcc_out.rearrange("(c j) f -> c j f", j=BPC)[:, j:j + 1, 0:HPC * D]
                        .squeeze(1)
                        .rearrange("c (hl d) -> c hl d", d=P)
                    )
                    nc.sync.dma_start(qj[:], src)
                    qjT_ps = tps.tile([P, P], dtb, tag="tps")
                    nc.tensor.transpose(qjT_ps[:], qj[:], idn[:])
                    qjT = spool.tile([P, P], dtb, tag="qjT")
                    nc.vector.tensor_copy(qjT[:], qjT_ps[:])
                    if debug:
                        nc.sync.dma_start(dbg_qjt.ap()[:, j * P:(j + 1) * P], qjT[:])

                    # staged caches for this slot
                    kt_sb = kvpool.tile([P, Tj * P], dtb, tag="kt")
                    nc.sync.dma_start(kt_sb[:], kt_d.ap()[:, toff[j] * P:(toff[j] + Tj) * P])
                    nc.vector.tensor_copy(kt_sb[:, 0:1], knewT[:, j:j + 1])
                    if debug:
                        nc.sync.dma_start(dbg_kt.ap()[:, j * P:(j + 1) * P], kt_sb[:, 0:P])
                    vv_sb = kvpool.tile([P, Tj * P], dtb, tag="vv")
                    nc.sync.dma_start(vv_sb[:], vv_d.ap()[:, toff[j] * P:(toff[j] + Tj) * P])
                    # new token sits at slot position 0 = partition 0 of v-tile 0
                    nc.vector.tensor_copy(vv_sb[0:1, 0:P], vnew[0:1, j * P:(j + 1) * P])

                    attn_ps = aps.tile([P, D_V], dtf, tag="attn")
                    sum_ps = sps.tile([P, 1], dtf, tag="sum")
                    for t in range(Tj):
                        sc_ps = scps.tile([P, P], dtf, tag="sc")
                        nc.tensor.matmul(
                            sc_ps[:], lhsT=kt_sb[:, t * P:(t + 1) * P], rhs=qjT[:],
                            start=True, stop=True,
                        )
                        et = epool.tile([P, P], dtb, tag="et")
                        nc.scalar.activation(
                            et[:], sc_ps[:], Exp,
                            bias=bias_sb[:, toff[j] + t:toff[j] + t + 1], scale=SCALE,
                        )
                        if debug:
                            nc.sync.dma_start(
                                dbg_et.ap()[:, (toff[j] + t) * P:(toff[j] + t + 1) * P],
                                et[:],
                            )
                        nc.tensor.matmul(
                            attn_ps[:], lhsT=et[:], rhs=vv_sb[:, t * P:(t + 1) * P],
                            start=(t == 0), stop=(t == Tj - 1), skip_group_check=True,
                        )
                        nc.tensor.matmul(
                            sum_ps[:], lhsT=et[:], rhs=ones[:],
                            start=(t == 0), stop=(t == Tj - 1), skip_group_check=True,
                        )
                    sum_sb = spool.tile([P, 1], dtf, tag="sumsb")
                    nc.vector.tensor_copy(sum_sb[:], sum_ps[:])
                    if debug:
                        nc.sync.dma_start(dbg_sum.ap()[:, j:j + 1], sum_sb[:])
                    recip = spool.tile([P, 1], dtf, tag="recip")
                    nc.vector.reciprocal(recip[:], sum_sb[:])
                    nc.scalar.activation(
                        attn_cc[:, j * D_V:(j + 1) * D_V], attn_ps[:], Copy,
                        bias=0.0, scale=recip[:],
                    )

            nc.sync.dma_start(acc_in[:], attn_cc[:])
            nc.gpsimd.collective_compute(
                "AllToAll", mybir.AluOpType.bypass, replica_groups=rg,
                ins=[acc_in.opt()], outs=[acc_out.opt()],
            )
            if debug:
                nc.sync.dma_start(dbg_at.ap(), attn_cc[:])
                nc.sync.dma_start(dbg_ao.ap(), acc_out[:])

            # ---- phase C: Wo partial (this core's 16 heads, all 128 batches) ----
            with (
                tc.tile_pool(name="tps2", bufs=2, space="PSUM") as tps2,
                tc.tile_pool(name="wops", bufs=2, space="PSUM") as wops,
            ):
                lhs_tiles = []
                for hl in range(HPC):
                    bm = spool.tile([P, P], dtb, tag="bm")
                    src = (
                        acc_out.rearrange("(c hl) f -> c hl f", hl=HPC)[:, hl:hl + 1, :]
                        .squeeze(1)
                        .rearrange("c (j d) -> c j d", d=P)
                    )
                    nc.sync.dma_start(bm[:], src)
                    lh_ps = tps2.tile([P, P], dtb, tag="tps2")
                    nc.tensor.transpose(lh_ps[:], bm[:], idn[:])
                    lh = lhpool.tile([P, P], dtb, tag="lh")
                    nc.vector.tensor_copy(lh[:], lh_ps[:])
                    lhs_tiles.append(lh)
                for (n0, nw) in WO_PASSES:
                    wo_ps = wops.tile([P, 1024], dtf, tag="wop")
                    for kt in range(HPC):
                        wt2 = wpool.tile([P, 1024], dtb, tag="wt2")
                        nc.sync.dma_start(wt2[:], wo_d.ap()[kt * P:(kt + 1) * P, n0:n0 + nw])
                        for half in range(2):
                            nc.tensor.matmul(
                                wo_ps[:, half * 512:(half + 1) * 512],
                                lhsT=lhs_tiles[kt][:],
                                rhs=wt2[:, half * 512:(half + 1) * 512],
                                start=(kt == 0), stop=(kt == HPC - 1),
                            )
                    out_sb = opool.tile([P, 1024], dtf, tag="osb")
                    nc.vector.tensor_copy(out_sb[:], wo_ps[:])
                    nc.sync.dma_start(out_d.ap()[:, n0:n0 + nw], out_sb[:])

    nc.compile()
    return nc


def kernel(hidden_states, k_cache, v_cache, Wq, Wkv, Wo, positions, slot_mapping,
           seq_lens):
    global LAST_RESULTS
    h = np.asarray(hidden_states, np.float32)[:, -1, :]        # [B, HID]
    k_cache = np.asarray(k_cache, np.float32)
    v_cache = np.asarray(v_cache, np.float32)
    Wq = np.asarray(Wq, np.float32)
    Wkv = np.asarray(Wkv, np.float32)
    Wo = np.asarray(Wo, np.float32)
    seq = np.asarray(seq_lens).astype(np.int64)

    # ---- schedule: sort batches by length, slot j holds ranks [8j, 8j+8) ----
    order = np.argsort(-seq, kind="stable")
    batch_of = np.empty((NC, BPC), np.int64)
    for j in range(BPC):
        for c in range(NC):
            batch_of[c, j] = order[NC * j + c]
    perm = batch_of.reshape(-1)                                # sigma (c-major)
    L = np.array([int(seq[order[NC * j]]) for j in range(BPC)])
    T = (L + P - 1) // P
    toff = np.concatenate([[0], np.cumsum(T)])[:-1].astype(np.int64)
    NTT = int(T.sum())

    key = (NTT, tuple(int(t) for t in T), os.environ.get("BASS_KERNEL_DEBUG", "0"))
    if key not in _PROGRAM_CACHE:
        _PROGRAM_CACHE.clear()
        _PROGRAM_CACHE[key] = _build_program(T, toff, NTT)
    nc = _PROGRAM_CACHE[key]

    # ---- host staging ----
    # h^T in device layout: [p, k*128 + b] = h_sigma[b, k*128+p]
    h_sigma = h[perm].astype(BF16)                             # [128, HID]
    ht_stage = np.ascontiguousarray(
        h_sigma.reshape(P, HID // P, P).transpose(2, 1, 0).reshape(P, HID)
    )
    idn_np = np.eye(P, dtype=BF16)
    one_np = np.ones((P, 1), BF16)

    in_maps = []
    for c in range(NC):
        cols = [Wq[:, (HPC * c + hl) * (D + D_ROPE):(HPC * c + hl) * (D + D_ROPE) + D]
                for hl in range(HPC)]
        wq_stage = np.concatenate(cols + [Wkv[:, :256]], axis=1).astype(BF16)
        wo_stage = np.ascontiguousarray(
            Wo[HPC * c * D_V:(HPC * c + HPC) * D_V, :]
        ).astype(BF16)

        kt_stage = np.zeros((P, NTT * P), BF16)
        vv_stage = np.zeros((P, NTT * P), BF16)
        bias_stage = np.full((P, NTT), -30000.0, np.float32)
        for j in range(BPC):
            b = int(batch_of[c, j])
            S = int(seq[b])
            o = int(toff[j]) * P
            # position 0 = new token (k patched on device, v handled by rank-1
            # matmul); positions 1..S-1 = cache rows 0..S-2
            kt_stage[:, o + 1:o + S] = k_cache[b, :S - 1, :].T.astype(BF16)
            vblk = np.zeros((int(T[j]) * P, P), BF16)
            vblk[1:S, :] = v_cache[b, :S - 1, :].astype(BF16)
            vv_stage[:, o:o + int(T[j]) * P] = (
                vblk.reshape(int(T[j]), P, P).transpose(1, 0, 2).reshape(P, -1)
            )
            pidx = np.arange(P)
            for t in range(int(T[j])):
                bias_stage[(t * P + pidx) < S, int(toff[j]) + t] = 0.0
        in_maps.append({
            "ht": ht_stage, "wq": wq_stage, "kt": kt_stage, "vv": vv_stage,
            "bias": bias_stage, "wo": wo_stage, "idn": idn_np, "one": one_np,
        })

    trace = os.environ.get("BASS_KERNEL_TRACE", "0") == "1"
    if trace:
        _install_ntff_hook()
    res = run_bass_kernel_spmd(nc, in_maps, core_ids=list(range(NC)), trace=trace)
    LAST_RESULTS = res
    global LAST_INMAPS, LAST_SCHED
    LAST_INMAPS = in_maps
    LAST_SCHED = (order, batch_of, perm, L, T, toff, NTT)

    out_sigma = np.zeros((P, HID), np.float32)
    for c in range(NC):
        out_sigma += res.results[c]["outp"]
    out_full = np.empty((B, HID), np.float32)
    out_full[perm] = out_sigma
    return out_full.reshape(B, 1, HID)



# revision 9
# speedup vs baseline: 1.3324x; 1.3324x over previous
"""DeepSeekV2-style MLA decode attention (MQA, B=128 decode tokens) on 8 trn2 NeuronCores.

Strategy (all shapes hardcoded from the problem spec):
  - Heads tensor-parallel for the projections: core c owns 16 heads (Wq cols /
    Wo rows sharded); Wkv replicated (tiny).
  - Batch-parallel for attention: core c owns 16 of the 128 decode batches;
    k/v caches are sharded by batch, truncated to seq_len, and staged by the
    host (kT pre-transposed, V with an appended 0/1 valid-mask column) so the
    device streams only valid positions and the softmax needs no bias masking:
    invalid positions get k=0 (=> exp(0)=1) but v=0 and mask=0, and the
    denominator is accumulated as column 128 of the AV matmul.
  - Two on-device AllToAlls reshard q (head-shard -> batch-shard) and attn
    output (batch-shard -> head-shard).  Final Wo partials (bf16) are summed
    on host.
  - New-token kv (h @ Wkv) is computed on-device; the new token occupies
    position 0 of each staged cache slot, so its patch location is static.
  - DMA queues: sync carries the Wq + cache streams, scalar carries bounces/
    gathers/output, gpsimd (SWDGE) carries the Wo stream ordered after the
    first AllToAll so it backfills bandwidth during attention.

The device program is specialized on seq_lens (static loop bounds); kernel()
re-builds/compiles whenever the derived schedule changes and memoizes it.
"""

import os
import numpy as np
import ml_dtypes

import concourse.bass as bass
import concourse.mybir as mybir
import concourse.tile as tile
from concourse import bacc
from concourse.bass_utils import run_bass_kernel_spmd

BF16 = ml_dtypes.bfloat16
P = 128
B, MAX_S, HID = 128, 4096, 5120
H, D, D_ROPE, D_V = 128, 128, 64, 128
NC, HPC, BPC = 8, 16, 16           # cores, heads/core, slots(batches)/core
QCOLS = HPC * D + 256              # 2304: per-core q cols + kv cols
SCALE = float(D) ** -0.5
PROJ_CHUNKS = [(0, 512), (512, 512), (1024, 512), (1536, 512), (2048, 256)]
GW = 4                             # s-tiles per exp/DMA chunk
VW = D_V + 1                       # 129: v tile width incl. mask column
CW = GW * (P + VW)                 # 1028: chunk cols (kt | vv)
NKSUP = HID // (2 * P)             # 20 wq super-tiles (2 k-tiles each)
WO_HALF = HID // 2                 # 2560

dtb = mybir.dt.bfloat16
dtf = mybir.dt.float32

_PROGRAM_CACHE: dict = {}
LAST_RESULTS = None


def _install_ntff_hook():
    """The agent image's ``antenv`` lacks ``axon_hooks``; recreate it so
    run_bass_kernel_spmd(trace=True) can capture NTFF profiles via the
    libaxon ctypes path. Best-effort — failure just means no trace."""
    import sys
    import types
    try:
        import antenv.axon_hooks  # noqa: F401
        return
    except ImportError:
        pass
    try:
        from trn_agent_boot.trn_boot import _ntff_profile_via_ctypes
        hook = _ntff_profile_via_ctypes("/opt/axon/libaxon_pjrt.so")
        mod = types.ModuleType("antenv.axon_hooks")
        mod._hook = hook
        mod.set_axon_ntff_profile_hook = lambda h: setattr(mod, "_hook", h)
        mod.get_axon_ntff_profile_hook = lambda: mod._hook
        sys.modules["antenv.axon_hooks"] = mod
        import antenv
        antenv.axon_hooks = mod
    except Exception:
        pass


def _chunks_of(Tj):
    """[(tile0, ntiles), ...] chunks of up to GW s-tiles for a slot."""
    out = []
    t = 0
    while t < Tj:
        w = min(GW, Tj - t)
        out.append((t, w))
        t += w
    return out


def _build_program(T, kvoff, KVCOLS, do_compile=True):
    """Build + compile the per-core bass program. T: per-slot tile counts;
    kvoff[j][g]: column offset of slot j's g-th kv chunk in the kv blob."""
    nc = bacc.Bacc("TRN2", target_bir_lowering=False, debug=False, num_devices=NC)

    ht_d = nc.dram_tensor("ht", [P, HID], dtb, kind="ExternalInput")
    wq_d = nc.dram_tensor("wq", [P, NKSUP * 2 * QCOLS], dtb, kind="ExternalInput")
    kv_d = nc.dram_tensor("kv", [P, KVCOLS], dtb, kind="ExternalInput")
    wo_d = nc.dram_tensor("wo", [HPC * D_V, HID], dtb, kind="ExternalInput")
    idn_d = nc.dram_tensor("idn", [P, P], dtb, kind="ExternalInput")
    out_d = nc.dram_tensor("outp", [P, HID], dtb, kind="ExternalOutput")

    rg = [list(range(NC))]
    Exp = mybir.ActivationFunctionType.Exp

    with tile.TileContext(nc) as tc:
        with (
            tc.tile_pool(name="cpool", bufs=1) as cpool,
            tc.tile_pool(name="wqpool", bufs=3) as wqpool,
            tc.tile_pool(name="kvpool", bufs=8) as kvpool,
            tc.tile_pool(name="qtpool", bufs=3) as qtpool,
            tc.tile_pool(name="epool", bufs=4) as epool,
            tc.tile_pool(name="spool", bufs=2) as spool,
            tc.tile_pool(name="lhpool", bufs=HPC) as lhpool,
            tc.tile_pool(name="wopool", bufs=16) as wopool,
            tc.tile_pool(name="opool", bufs=2) as opool,
            tc.tile_pool(name="dram", bufs=1, space="DRAM") as dram,
        ):
            # ---- constants / global loads ----
            idn = cpool.tile([P, P], dtb)
            nc.sync.dma_start(idn[:], idn_d.ap())
            hts = cpool.tile([P, HID], dtb)
            nc.sync.dma_start(hts[:], ht_d.ap())

            qcc_in = dram.tile([P, QCOLS], dtb)
            qcc_out = dram.tile([P, QCOLS], dtb)
            acc_in = dram.tile([P, BPC * D_V], dtb)
            acc_out = dram.tile([P, BPC * D_V], dtb)

            # ---- phase A: q/kv projection (this core's 16 heads, all 128 batches) ----
            qc_sb = cpool.tile([P, QCOLS], dtb)
            with tc.tile_pool(name="pjp", bufs=1, space="PSUM") as pjp:
                q_ps = pjp.tile([P, QCOLS], dtf)
                for s in range(NKSUP):
                    wt = wqpool.tile([P, 2 * QCOLS], dtb, tag="wt")
                    nc.sync.dma_start(wt[:], wq_d.ap()[:, s * 2 * QCOLS:(s + 1) * 2 * QCOLS])
                    for half in range(2):
                        k = 2 * s + half
                        for (c0, cw) in PROJ_CHUNKS:
                            nc.tensor.matmul(
                                q_ps[:, c0:c0 + cw],
                                lhsT=hts[:, k * P:(k + 1) * P],
                                rhs=wt[:, half * QCOLS + c0:half * QCOLS + c0 + cw],
                                start=(k == 0),
                                stop=(k == 2 * NKSUP - 1),
                            )
                # copy per chunk so the tail overlaps the last matmuls
                for (c0, cw) in PROJ_CHUNKS:
                    nc.vector.tensor_copy(qc_sb[:, c0:c0 + cw], q_ps[:, c0:c0 + cw])

            nc.scalar.dma_start(qcc_in[:], qc_sb[:])
            nc.gpsimd.collective_compute(
                "AllToAll", mybir.AluOpType.bypass, replica_groups=rg,
                ins=[qcc_in.opt()], outs=[qcc_out.opt()],
            )

            # ---- phase C weights, first half: emitted here so the gpsimd queue
            # orders them after the A2A1 trigger (no front-running the Wq load).
            wo_tiles = []
            for kt in range(HPC):
                wt2 = wopool.tile([P, WO_HALF], dtb, tag="wt2")
                nc.gpsimd.dma_start(wt2[:], wo_d.ap()[kt * P:(kt + 1) * P, 0:WO_HALF])
                wo_tiles.append(wt2)

            # ---- phase B: attention over this core's 16 batches ----
            attn_cc = cpool.tile([P, BPC * D_V], dtb)
            with (
                tc.tile_pool(name="tps", bufs=2, space="PSUM") as tps,
                tc.tile_pool(name="scps", bufs=3, space="PSUM") as scps,
                tc.tile_pool(name="aps", bufs=2, space="PSUM") as aps,
            ):
                # gather all 16 slots' q rows in one strided DMA:
                # qj_all[(c,hl), (j,d)] = qcc_out[c*16+j, hl*128+d]
                qj_all = cpool.tile([P, BPC * P], dtb)
                for cc in range(NC):
                    nc.scalar.dma_start(
                        qj_all[cc * HPC:(cc + 1) * HPC, :]
                        .rearrange("hl (j d) -> hl j d", d=P),
                        qcc_out.rearrange("(c j) (hl d) -> c hl j d", j=BPC, d=P)[cc, 0:HPC],
                    )
                # new-token k/v from the AllToAll'd kv columns (identical from
                # every source core; read source block 0)
                knew = cpool.tile([BPC, P], dtb)
                nc.scalar.dma_start(knew[:], qcc_out[0:BPC, HPC * D:HPC * D + P])
                knewT_ps = tps.tile([P, BPC], dtb, tag="tps")
                nc.tensor.transpose(knewT_ps[:], knew[:], idn[0:BPC, 0:BPC])
                knewT = cpool.tile([P, BPC], dtb)
                nc.vector.tensor_copy(knewT[:], knewT_ps[:])
                vnew = cpool.tile([1, BPC * P], dtb)
                nc.scalar.dma_start(
                    vnew.rearrange("one (j d) -> one j d", d=P),
                    qcc_out.rearrange("(c j) f -> c j f", j=BPC)[0:1, :, HPC * D + P:QCOLS],
                )

                for j in range(BPC):
                    Tj = int(T[j])
                    # q_j [128 h, 128 d] -> qjT [128 d, 128 h]
                    qjT_ps = tps.tile([P, P], dtb, tag="tps")
                    nc.tensor.transpose(qjT_ps[:], qj_all[:, j * P:(j + 1) * P], idn[:])
                    qjT = qtpool.tile([P, P], dtb, tag="qjT")
                    nc.vector.tensor_copy(qjT[:], qjT_ps[:])

                    attn_ps = aps.tile([P, VW], dtf, tag="attn")
                    for (t0, w) in _chunks_of(Tj):
                        kvt = kvpool.tile([P, CW], dtb, tag="kv")
                        off = kvoff[j][t0 // GW]
                        nc.sync.dma_start(
                            kvt[:, 0:w * (P + VW)],
                            kv_d.ap()[:, off:off + w * (P + VW)],
                        )
                        if t0 == 0:
                            # patch new-token k (column 0 of kt) and v (row 0
                            # of vv values; its mask col is host-staged 1.0)
                            nc.vector.tensor_copy(kvt[:, 0:1], knewT[:, j:j + 1])
                            nc.vector.tensor_copy(
                                kvt[0:1, w * P:w * P + D_V],
                                vnew[0:1, j * P:j * P + D_V],
                            )
                        sc = scps.tile([P, GW * P], dtf, tag="sc")
                        for l in range(w):
                            nc.tensor.matmul(
                                sc[:, l * P:(l + 1) * P],
                                lhsT=kvt[:, l * P:(l + 1) * P], rhs=qjT[:],
                                start=True, stop=True,
                            )
                        et = epool.tile([P, GW * P], dtb, tag="et")
                        nc.scalar.activation(
                            et[:, 0:w * P], sc[:, 0:w * P], Exp,
                            bias=0.0, scale=SCALE,
                        )
                        for l in range(w):
                            t = t0 + l
                            nc.tensor.matmul(
                                attn_ps[:],
                                lhsT=et[:, l * P:(l + 1) * P],
                                rhs=kvt[:, w * P + l * VW:w * P + (l + 1) * VW],
                                start=(t == 0), stop=(t == Tj - 1),
                                skip_group_check=True,
                            )
                    recip = spool.tile([P, 1], dtf, tag="recip")
                    nc.vector.reciprocal(recip[:], attn_ps[:, D_V:D_V + 1])
                    nc.vector.tensor_scalar_mul(
                        attn_cc[:, j * D_V:(j + 1) * D_V], attn_ps[:, 0:D_V], recip[:],
                    )

            nc.scalar.dma_start(acc_in[:], attn_cc[:])
            nc.gpsimd.collective_compute(
                "AllToAll", mybir.AluOpType.bypass, replica_groups=rg,
                ins=[acc_in.opt()], outs=[acc_out.opt()],
            )
            # second half of Wo: after the A2A2 trigger on the gpsimd queue
            # (slots only free once pass-1 matmuls consume them).
            for kt in range(HPC):
                wt2 = wopool.tile([P, WO_HALF], dtb, tag="wt2")
                nc.gpsimd.dma_start(wt2[:], wo_d.ap()[kt * P:(kt + 1) * P, WO_HALF:HID])
                wo_tiles.append(wt2)

            # ---- phase C: Wo partial (this core's 16 heads, all 128 batches) ----
            with (
                tc.tile_pool(name="tps2", bufs=2, space="PSUM") as tps2,
                tc.tile_pool(name="wops", bufs=1, space="PSUM") as wops,
            ):
                # bm_all[(c,j), (hl,d)] = acc_out[c*16+hl, j*128+d]
                bm_all = cpool.tile([P, HPC * P], dtb)
                for cc in range(NC):
                    nc.scalar.dma_start(
                        bm_all[cc * BPC:(cc + 1) * BPC, :]
                        .rearrange("j (hl d) -> j hl d", d=P),
                        acc_out.rearrange("(c hl) (j d) -> c j hl d", hl=HPC, d=P)[cc],
                    )
                lhs_tiles = []
                for hl in range(HPC):
                    lh_ps = tps2.tile([P, P], dtb, tag="tps2")
                    nc.tensor.transpose(lh_ps[:], bm_all[:, hl * P:(hl + 1) * P], idn[:])
                    lh = lhpool.tile([P, P], dtb, tag="lh")
                    nc.vector.tensor_copy(lh[:], lh_ps[:])
                    lhs_tiles.append(lh)
                for half in range(2):
                    wo_ps = wops.tile([P, WO_HALF], dtf, tag="wop")
                    for kt in range(HPC):
                        wt2 = wo_tiles[half * HPC + kt]
                        for c in range(WO_HALF // 512):
                            nc.tensor.matmul(
                                wo_ps[:, c * 512:(c + 1) * 512],
                                lhsT=lhs_tiles[kt][:],
                                rhs=wt2[:, c * 512:(c + 1) * 512],
                                start=(kt == 0), stop=(kt == HPC - 1),
                            )
                    out_sb = opool.tile([P, WO_HALF], dtb, tag="osb")
                    nc.vector.tensor_copy(out_sb[:], wo_ps[:])
                    nc.scalar.dma_start(
                        out_d.ap()[:, half * WO_HALF:(half + 1) * WO_HALF], out_sb[:],
                    )

    if do_compile:
        nc.compile()
    return nc


def kernel(hidden_states, k_cache, v_cache, Wq, Wkv, Wo, positions, slot_mapping,
           seq_lens):
    global LAST_RESULTS
    h = np.asarray(hidden_states, np.float32)[:, -1, :]        # [B, HID]
    k_cache = np.asarray(k_cache, np.float32)
    v_cache = np.asarray(v_cache, np.float32)
    Wq = np.asarray(Wq, np.float32)
    Wkv = np.asarray(Wkv, np.float32)
    Wo = np.asarray(Wo, np.float32)
    seq = np.asarray(seq_lens).astype(np.int64)

    # ---- schedule: sort batches by length, slot j holds ranks [8j, 8j+8) ----
    order = np.argsort(-seq, kind="stable")
    batch_of = np.empty((NC, BPC), np.int64)
    for j in range(BPC):
        for c in range(NC):
            batch_of[c, j] = order[NC * j + c]
    perm = batch_of.reshape(-1)                                # sigma (c-major)
    L = np.array([int(seq[order[NC * j]]) for j in range(BPC)])
    T = (L + P - 1) // P

    # kv blob layout: per slot, chunks of up to GW tiles; chunk = kt|vv cols
    kvoff = []
    KVCOLS = 0
    for j in range(BPC):
        offs = []
        for (t0, w) in _chunks_of(int(T[j])):
            offs.append(KVCOLS)
            KVCOLS += w * (P + VW)
        kvoff.append(offs)

    key = (KVCOLS, tuple(int(t) for t in T))
    if key not in _PROGRAM_CACHE:
        _PROGRAM_CACHE.clear()
        _PROGRAM_CACHE[key] = _build_program(T, kvoff, KVCOLS)
    nc = _PROGRAM_CACHE[key]

    # ---- host staging ----
    # h^T in device layout: [p, k*128 + b] = h_sigma[b, k*128+p]
    h_sigma = h[perm].astype(BF16)                             # [128, HID]
    ht_stage = np.ascontiguousarray(
        h_sigma.reshape(P, HID // P, P).transpose(2, 1, 0).reshape(P, HID)
    )
    idn_np = np.eye(P, dtype=BF16)

    in_maps = []
    for c in range(NC):
        cols = [Wq[:, (HPC * c + hl) * (D + D_ROPE):(HPC * c + hl) * (D + D_ROPE) + D]
                for hl in range(HPC)]
        wq_flat = np.concatenate(cols + [Wkv[:, :256]], axis=1).astype(BF16)  # [5120, 2304]
        # pack 2 k-tiles per super-tile: [128, 20*2*2304],
        # col = s*2*QCOLS + half*QCOLS + q, value = wq_flat[(2s+half)*128 + p, q]
        wq_stage = np.ascontiguousarray(
            wq_flat.reshape(NKSUP, 2, P, QCOLS)
            .transpose(2, 0, 1, 3)
            .reshape(P, NKSUP * 2 * QCOLS)
        )
        wo_stage = np.ascontiguousarray(
            Wo[HPC * c * D_V:(HPC * c + HPC) * D_V, :]
        ).astype(BF16)

        kv_stage = np.zeros((P, KVCOLS), BF16)
        for j in range(BPC):
            b = int(batch_of[c, j])
            S = int(seq[b])
            Tj = int(T[j])
            # kt block [128 d, Tj*128 s]: position 0 = new token (patched on
            # device), positions 1..S-1 = cache rows 0..S-2, rest 0.
            ktb = np.zeros((P, Tj * P), BF16)
            ktb[:, 1:S] = k_cache[b, :S - 1, :].T.astype(BF16)
            # vv block tiles [128 s, 129]: col 128 = valid mask
            vblk = np.zeros((Tj * P, VW), BF16)
            vblk[1:S, :D_V] = v_cache[b, :S - 1, :].astype(BF16)
            vblk[:S, D_V] = 1.0
            vtiles = vblk.reshape(Tj, P, VW)
            for gi, (t0, w) in enumerate(_chunks_of(Tj)):
                off = kvoff[j][gi]
                kw = w * P
                kv_stage[:, off:off + kw] = ktb[:, t0 * P:(t0 + w) * P]
                kv_stage[:, off + kw:off + kw + w * VW] = (
                    vtiles[t0:t0 + w].transpose(1, 0, 2).reshape(P, w * VW)
                )
        in_maps.append({
            "ht": ht_stage, "wq": wq_stage, "kv": kv_stage,
            "wo": wo_stage, "idn": idn_np,
        })

    trace = os.environ.get("BASS_KERNEL_TRACE", "0") == "1"
    if trace:
        _install_ntff_hook()
    res = run_bass_kernel_spmd(nc, in_maps, core_ids=list(range(NC)), trace=trace)
    LAST_RESULTS = res

    out_sigma = np.zeros((P, HID), np.float32)
    for c in range(NC):
        out_sigma += res.results[c]["outp"].astype(np.float32)
    out_full = np.empty((B, HID), np.float32)
    out_full[perm] = out_sigma
    return out_full.reshape(B, 1, HID)
